# revision 6
# baseline (speedup 1.0000x reference)
"""TRN2 Bass kernel for nn_EnsemblePointNet: 1296 independent 4-layer MLPs.

Strategy: shard the model dim (1296 -> 162 per core) across 8 NeuronCores.
Per model, activations live transposed ([feature, batch]) so every layer is
one PE matmul with the stored weights as lhsT:
    z = W.T @ h   (lhsT=W [K,M], rhs=h_T [K,B])
fp32r matmuls (full PE rate, ~1.5e-4 rel err). Bias+ReLU fused into the
PSUM->SBUF eviction pass, split between the Scalar (ACT) and Vector (DVE)
engines. The [1,B] layer-3 outputs are evicted to a partition-0 scratch row,
DMA'd into a 128-model gather tile, bias-added there, and stored contiguously.
"""

import contextlib
import sys

sys.path.insert(0, "/opt/trn_rl_repo")

import numpy as np

import concourse.bass as bass
import concourse.mybir as mybir
import concourse.tile as tile
from concourse import bacc
from concourse.bass_utils import run_bass_kernel_spmd

F32 = mybir.dt.float32
F32R = mybir.dt.float32r
AF = mybir.ActivationFunctionType
OP = mybir.AluOpType

M_TOT = 1296
N_CORES = 8
M_LOC = M_TOT // N_CORES  # 162
B = 1024
DIN = 8
H = 128

# ACT is 1.2 GHz, DVE 0.96 GHz; split each relu pass proportionally.
ACT_COLS = 576  # of 1024
HB = 512  # psum bank width in fp32


def build_nc(m_loc=M_LOC, loop_n=1):
    nc = bacc.Bacc("TRN2", target_bir_lowering=False, debug=False)
    xt = nc.dram_tensor("xt", [m_loc, DIN, B], F32, kind="ExternalInput").ap()
    w0 = nc.dram_tensor("w0", [m_loc, DIN, H], F32, kind="ExternalInput").ap()
    w1 = nc.dram_tensor("w1", [m_loc, H, H], F32, kind="ExternalInput").ap()
    w2 = nc.dram_tensor("w2", [m_loc, H, H], F32, kind="ExternalInput").ap()
    w3t = nc.dram_tensor("w3t", [H, m_loc], F32, kind="ExternalInput").ap()
    b0t = nc.dram_tensor("b0t", [H, m_loc], F32, kind="ExternalInput").ap()
    b1t = nc.dram_tensor("b1t", [H, m_loc], F32, kind="ExternalInput").ap()
    b2t = nc.dram_tensor("b2t", [H, m_loc], F32, kind="ExternalInput").ap()
    ngrp = (m_loc + 127) // 128
    b3t = nc.dram_tensor("b3t", [128, ngrp], F32, kind="ExternalInput").ap()
    y = nc.dram_tensor("y", [m_loc, B], F32, kind="ExternalOutput").ap()

    with tile.TileContext(nc) as tc:
        with (
            tc.tile_pool(name="consts", bufs=1) as consts,
            tc.tile_pool(name="wpool", bufs=4) as wpool,
            tc.tile_pool(name="xpool", bufs=4) as xpool,
            tc.tile_pool(name="hpool", bufs=2) as hpool,
            tc.tile_pool(name="ypool", bufs=2) as ypool,
            tc.tile_pool(name="spool", bufs=4) as spool,
            tc.tile_pool(name="zpool", bufs=3, space="PSUM") as zpool,
            tc.tile_pool(name="z3pool", bufs=2, space="PSUM") as z3pool,
        ):
            # one-time constants
            w3t_s = consts.tile([H, m_loc], F32R)
            nc.sync.dma_start(out=w3t_s, in_=w3t.bitcast(F32R))
            b0t_s = consts.tile([H, m_loc], F32)
            nc.sync.dma_start(out=b0t_s, in_=b0t)
            b1t_s = consts.tile([H, m_loc], F32)
            nc.sync.dma_start(out=b1t_s, in_=b1t)
            b2t_s = consts.tile([H, m_loc], F32)
            nc.sync.dma_start(out=b2t_s, in_=b2t)
            b3t_s = consts.tile([128, ngrp], F32)
            nc.sync.dma_start(out=b3t_s, in_=b3t)

            def relu_pass(dst, zsrc, bias_ap):
                # dst[128, B] sbuf F32R <- relu(zsrc[128, B] psum + bias)
                nc.scalar.activation(
                    dst[:, 0:ACT_COLS], zsrc[:, 0:ACT_COLS], AF.Relu,
                    bias=bias_ap, scale=1.0,
                )
                nc.vector.tensor_scalar(
                    out=dst[:, ACT_COLS:B], in0=zsrc[:, ACT_COLS:B],
                    scalar1=bias_ap, scalar2=0.0, op0=OP.add, op1=OP.max,
                )

            def body():
                for g in range(ngrp):
                    g0 = g * 128
                    gm = min(128, m_loc - g0)
                    ygat = ypool.tile([128, B], F32, tag="ygat")
                    for mi in range(gm):
                        m = g0 + mi
                        # per-model loads
                        w0s = wpool.tile([DIN, H], F32R, tag="w0")
                        w1s = wpool.tile([H, H], F32R, tag="w1")
                        w2s = wpool.tile([H, H], F32R, tag="w2")
                        xts = xpool.tile([DIN, B], F32R, tag="xt")
                        nc.sync.dma_start(out=w0s, in_=w0[m].bitcast(F32R))
                        nc.sync.dma_start(out=w1s, in_=w1[m].bitcast(F32R))
                        nc.sync.dma_start(out=w2s, in_=w2[m].bitcast(F32R))
                        nc.sync.dma_start(out=xts, in_=xt[m].bitcast(F32R))

                        # L0: z0 = W0.T @ x_T
                        z0 = zpool.tile([H, B], F32, tag="z")
                        nc.tensor.matmul(z0[:, 0:HB], w0s, xts[:, 0:HB], start=True, stop=True)
                        nc.tensor.matmul(z0[:, HB:B], w0s, xts[:, HB:B], start=True, stop=True)
                        h1 = hpool.tile([H, B], F32R, tag="h1")
                        relu_pass(h1, z0, b0t_s[:, m : m + 1])

                        # L1
                        z1 = zpool.tile([H, B], F32, tag="z")
                        nc.tensor.matmul(z1[:, 0:HB], w1s, h1[:, 0:HB], start=True, stop=True)
                        nc.tensor.matmul(z1[:, HB:B], w1s, h1[:, HB:B], start=True, stop=True)
                        h2 = hpool.tile([H, B], F32R, tag="h2")
                        relu_pass(h2, z1, b1t_s[:, m : m + 1])

                        # L2
                        z2 = zpool.tile([H, B], F32, tag="z")
                        nc.tensor.matmul(z2[:, 0:HB], w2s, h2[:, 0:HB], start=True, stop=True)
                        nc.tensor.matmul(z2[:, HB:B], w2s, h2[:, HB:B], start=True, stop=True)
                        h3 = hpool.tile([H, B], F32R, tag="h3")
                        relu_pass(h3, z2, b2t_s[:, m : m + 1])

                        # L3: z3 = w3.T @ h3 -> [1, B] in bank-sized halves
                        z3a = z3pool.tile([1, HB], F32, tag="z3")
                        z3b = z3pool.tile([1, HB], F32, tag="z3")
                        nc.tensor.matmul(z3a, w3t_s[:, m : m + 1], h3[:, 0:HB], start=True, stop=True)
                        nc.tensor.matmul(z3b, w3t_s[:, m : m + 1], h3[:, HB:B], start=True, stop=True)
                        # evict [1,B] psum to a partition-0 scratch row (engines
                        # are lane-locked), then DMA the row into the gather tile.
                        scr = spool.tile([1, B], F32, tag="scr")
                        nc.scalar.copy(out=scr[:, 0:HB], in_=z3a)
                        nc.vector.tensor_copy(out=scr[:, HB:B], in_=z3b)
                        nc.sync.dma_start(out=ygat[mi : mi + 1, :], in_=scr)

                    # bias add + store for the group
                    yout = ypool.tile([128, B], F32, tag="yout")
                    nc.vector.tensor_scalar(
                        out=yout[0:gm], in0=ygat[0:gm],
                        scalar1=b3t_s[0:gm, g : g + 1], scalar2=None, op0=OP.add,
                    )
                    nc.sync.dma_start(out=y[g0 : g0 + gm, :], in_=yout[0:gm])

            if loop_n > 1:
                with tc.For_i(0, loop_n, 1):
                    body()
            else:
                body()

    nc.compile()
    return nc


_NC_CACHE = {}


def _get_nc(m_loc):
    if m_loc not in _NC_CACHE:
        _NC_CACHE[m_loc] = build_nc(m_loc)
    return _NC_CACHE[m_loc]


def _prep_core_inputs(x, W0, b0, W1, b1, W2, b2, W3, b3, sl):
    m_loc = sl.stop - sl.start
    ngrp = (m_loc + 127) // 128
    xt = np.ascontiguousarray(np.transpose(x[sl], (0, 2, 1)))  # [m, DIN, B]
    b3_pad = np.zeros((ngrp * 128,), np.float32)
    b3_pad[:m_loc] = b3[sl, 0]
    return {
        "xt": xt,
        "w0": np.ascontiguousarray(W0[sl]),
        "w1": np.ascontiguousarray(W1[sl]),
        "w2": np.ascontiguousarray(W2[sl]),
        "w3t": np.ascontiguousarray(W3[sl, :, 0].T),  # [H, m]
        "b0t": np.ascontiguousarray(b0[sl].T),
        "b1t": np.ascontiguousarray(b1[sl].T),
        "b2t": np.ascontiguousarray(b2[sl].T),
        "b3t": np.ascontiguousarray(b3_pad.reshape(ngrp, 128).T),
    }


def kernel(x, W0, b0, W1, b1, W2, b2, W3, b3):
    x = np.asarray(x, dtype=np.float32)
    W0 = np.asarray(W0, np.float32); b0 = np.asarray(b0, np.float32)
    W1 = np.asarray(W1, np.float32); b1 = np.asarray(b1, np.float32)
    W2 = np.asarray(W2, np.float32); b2 = np.asarray(b2, np.float32)
    W3 = np.asarray(W3, np.float32); b3 = np.asarray(b3, np.float32)

    m_tot = x.shape[0]
    m_loc = m_tot // N_CORES
    nc = _get_nc(m_loc)
    in_maps = [
        _prep_core_inputs(x, W0, b0, W1, b1, W2, b2, W3, b3,
                          slice(c * m_loc, (c + 1) * m_loc))
        for c in range(N_CORES)
    ]
    res = run_bass_kernel_spmd(nc, in_maps, core_ids=list(range(N_CORES)))
    out = np.concatenate([r["y"] for r in res.results], axis=0)
    return out.reshape(m_tot, B, 1).astype(np.float32)


# revision 11
# speedup vs baseline: 1.0908x; 1.0908x over previous
"""TRN2 Bass kernel for nn_EnsemblePointNet: 1296 independent 4-layer MLPs.

Strategy: shard the model dim (1296 -> 162 per core) across 8 NeuronCores.
Per model, activations live transposed ([feature, batch]) so every layer is
one PE matmul with the stored weights as lhsT:
    z = W.T @ h   (lhsT=W [K,M], rhs=h_T [K,B])
fp32r matmuls (full PE rate, ~1.5e-4 rel err). Bias+ReLU fused into the
PSUM->SBUF eviction pass, split between the Scalar (ACT) and Vector (DVE)
engines. DMAs are batched across models (per-DMA fixed cost ~1us dominates
otherwise). The [1,B] layer-3 outputs are evicted to a partition-0 scratch
row shared by XGRP models, DMA'd into a 128-model gather tile, bias-added
there, and stored contiguously.
"""

import contextlib
import sys

sys.path.insert(0, "/opt/trn_rl_repo")

import numpy as np

import concourse.bass as bass
import concourse.mybir as mybir
import concourse.tile as tile
from concourse import bacc
from concourse.bass_utils import run_bass_kernel_spmd

F32 = mybir.dt.float32
F32R = mybir.dt.float32r
AF = mybir.ActivationFunctionType
OP = mybir.AluOpType

M_TOT = 1296
N_CORES = 8
M_LOC = M_TOT // N_CORES  # 162
B = 1024
DIN = 8
H = 128

# ACT is 1.2 GHz, DVE 0.96 GHz; split each relu pass proportionally.
ACT_COLS = 576  # of 1024
HB = 512  # psum bank width in fp32

WGRP = 3   # models per w12 load (w1+w2 packed, 128KB each)
XGRP = 6   # models per xt load / scr evict row
W0GRP = 27  # models per w0 load
GRP = 54   # models per output gather group


def build_nc(m_loc=M_LOC, loop_n=1):
    assert m_loc % GRP == 0 and GRP % XGRP == 0 and GRP % WGRP == 0 and GRP % W0GRP == 0
    nc = bacc.Bacc("TRN2", target_bir_lowering=False, debug=False)
    xt = nc.dram_tensor("xt", [m_loc, DIN, B], F32, kind="ExternalInput").ap()
    w0 = nc.dram_tensor("w0", [m_loc, DIN, H], F32, kind="ExternalInput").ap()
    w12 = nc.dram_tensor("w12", [m_loc, 2, H, H], F32, kind="ExternalInput").ap()
    w3t = nc.dram_tensor("w3t", [H, m_loc], F32, kind="ExternalInput").ap()
    b0t = nc.dram_tensor("b0t", [H, m_loc], F32, kind="ExternalInput").ap()
    b1t = nc.dram_tensor("b1t", [H, m_loc], F32, kind="ExternalInput").ap()
    b2t = nc.dram_tensor("b2t", [H, m_loc], F32, kind="ExternalInput").ap()
    ngrp = m_loc // GRP
    b3t = nc.dram_tensor("b3t", [GRP, ngrp], F32, kind="ExternalInput").ap()
    y = nc.dram_tensor("y", [m_loc, B], F32, kind="ExternalOutput").ap()

    with tile.TileContext(nc) as tc:
        with (
            tc.tile_pool(name="consts", bufs=1) as consts,
            tc.tile_pool(name="wpool", bufs=3) as wpool,
            tc.tile_pool(name="w0pool", bufs=2) as w0pool,
            tc.tile_pool(name="xpool", bufs=2) as xpool,
            tc.tile_pool(name="hpool", bufs=2) as hpool,
            tc.tile_pool(name="ypool", bufs=2) as ypool,
            tc.tile_pool(name="spool", bufs=2) as spool,
            tc.tile_pool(name="zpool", bufs=3, space="PSUM") as zpool,
            tc.tile_pool(name="z3pool", bufs=2, space="PSUM") as z3pool,
        ):
            # one-time constants
            w3t_s = consts.tile([H, m_loc], F32R)
            nc.sync.dma_start(out=w3t_s, in_=w3t.bitcast(F32R))
            b0t_s = consts.tile([H, m_loc], F32)
            nc.sync.dma_start(out=b0t_s, in_=b0t)
            b1t_s = consts.tile([H, m_loc], F32)
            nc.sync.dma_start(out=b1t_s, in_=b1t)
            b2t_s = consts.tile([H, m_loc], F32)
            nc.sync.dma_start(out=b2t_s, in_=b2t)
            b3t_s = consts.tile([GRP, ngrp], F32)
            nc.sync.dma_start(out=b3t_s, in_=b3t)

            def relu_pass(dst, zsrc, bias_ap):
                # dst[128, B] sbuf F32R <- relu(zsrc[128, B] psum + bias)
                nc.scalar.activation(
                    dst[:, 0:ACT_COLS], zsrc[:, 0:ACT_COLS], AF.Relu,
                    bias=bias_ap, scale=1.0,
                )
                nc.vector.tensor_scalar(
                    out=dst[:, ACT_COLS:B], in0=zsrc[:, ACT_COLS:B],
                    scalar1=bias_ap, scalar2=0.0, op0=OP.add, op1=OP.max,
                )

            def body():
                w12s = xts = w0s = scrw = ygat = None
                for g in range(ngrp):
                    g0 = g * GRP
                    gm = GRP
                    ygat = ypool.tile([GRP, B], F32, tag="ygat")
                    for mi in range(gm):
                        m = g0 + mi
                        # batched loads
                        if m % W0GRP == 0:
                            w0s = w0pool.tile([DIN, W0GRP * H], F32R, tag="w0")
                            nc.sync.dma_start(
                                out=w0s,
                                in_=w0[m : m + W0GRP].rearrange("m i h -> i m h").bitcast(F32R),
                            )
                        if m % WGRP == 0:
                            w12s = wpool.tile([H, WGRP * 2 * H], F32R, tag="w12")
                            nc.sync.dma_start(
                                out=w12s,
                                in_=w12[m : m + WGRP].rearrange("m l h k -> h m l k").bitcast(F32R),
                            )
                        if m % XGRP == 0:
                            xts = xpool.tile([DIN, XGRP * B], F32R, tag="xt")
                            nc.sync.dma_start(
                                out=xts,
                                in_=xt[m : m + XGRP].rearrange("m i b -> i m b").bitcast(F32R),
                            )
                            scrw = spool.tile([1, XGRP * B], F32, tag="scr")
                        xo = (m % XGRP) * B
                        wo = (m % WGRP) * 2 * H
                        w0o = (m % W0GRP) * H

                        # L0: z0 = W0.T @ x_T
                        z0 = zpool.tile([H, B], F32, tag="z")
                        nc.tensor.matmul(z0[:, 0:HB], w0s[:, w0o : w0o + H], xts[:, xo : xo + HB], start=True, stop=True)
                        nc.tensor.matmul(z0[:, HB:B], w0s[:, w0o : w0o + H], xts[:, xo + HB : xo + B], start=True, stop=True)
                        h1 = hpool.tile([H, B], F32R, tag="h1")
                        relu_pass(h1, z0, b0t_s[:, m : m + 1])

                        # L1
                        z1 = zpool.tile([H, B], F32, tag="z")
                        nc.tensor.matmul(z1[:, 0:HB], w12s[:, wo : wo + H], h1[:, 0:HB], start=True, stop=True)
                        nc.tensor.matmul(z1[:, HB:B], w12s[:, wo : wo + H], h1[:, HB:B], start=True, stop=True)
                        h2 = hpool.tile([H, B], F32R, tag="h2")
                        relu_pass(h2, z1, b1t_s[:, m : m + 1])

                        # L2
                        z2 = zpool.tile([H, B], F32, tag="z")
                        nc.tensor.matmul(z2[:, 0:HB], w12s[:, wo + H : wo + 2 * H], h2[:, 0:HB], start=True, stop=True)
                        nc.tensor.matmul(z2[:, HB:B], w12s[:, wo + H : wo + 2 * H], h2[:, HB:B], start=True, stop=True)
                        h3 = hpool.tile([H, B], F32R, tag="h3")
                        relu_pass(h3, z2, b2t_s[:, m : m + 1])

                        # L3: z3 = w3.T @ h3 -> [1, B] in bank-sized halves
                        z3a = z3pool.tile([1, HB], F32, tag="z3")
                        z3b = z3pool.tile([1, HB], F32, tag="z3")
                        nc.tensor.matmul(z3a, w3t_s[:, m : m + 1], h3[:, 0:HB], start=True, stop=True)
                        nc.tensor.matmul(z3b, w3t_s[:, m : m + 1], h3[:, HB:B], start=True, stop=True)
                        # evict [1,B] psum into the shared scratch row
                        nc.scalar.copy(out=scrw[:, xo : xo + HB], in_=z3a)
                        nc.vector.tensor_copy(out=scrw[:, xo + HB : xo + B], in_=z3b)
                        if m % XGRP == XGRP - 1:
                            nc.sync.dma_start(
                                out=ygat[mi - XGRP + 1 : mi + 1, :], in_=scrw
                            )

                    # bias add + store for the group
                    yout = ypool.tile([GRP, B], F32, tag="yout")
                    nc.vector.tensor_scalar(
                        out=yout[0:gm], in0=ygat[0:gm],
                        scalar1=b3t_s[0:gm, g : g + 1], scalar2=None, op0=OP.add,
                    )
                    nc.sync.dma_start(out=y[g0 : g0 + gm, :], in_=yout[0:gm])

            if loop_n > 1:
                with tc.For_i(0, loop_n, 1):
                    body()
            else:
                body()

    nc.compile()
    return nc


_NC_CACHE = {}


def _get_nc(m_loc):
    if m_loc not in _NC_CACHE:
        _NC_CACHE[m_loc] = build_nc(m_loc)
    return _NC_CACHE[m_loc]


def _prep_core_inputs(x, W0, b0, W1, b1, W2, b2, W3, b3, sl):
    m_loc = sl.stop - sl.start
    ngrp = m_loc // GRP
    xt = np.ascontiguousarray(np.transpose(x[sl], (0, 2, 1)))  # [m, DIN, B]
    w12 = np.ascontiguousarray(
        np.stack([W1[sl], W2[sl]], axis=1)  # [m, 2, H, H]
    )
    b3_pad = b3[sl, 0].astype(np.float32)
    return {
        "xt": xt,
        "w0": np.ascontiguousarray(W0[sl]),
        "w12": w12,
        "w3t": np.ascontiguousarray(W3[sl, :, 0].T),  # [H, m]
        "b0t": np.ascontiguousarray(b0[sl].T),
        "b1t": np.ascontiguousarray(b1[sl].T),
        "b2t": np.ascontiguousarray(b2[sl].T),
        "b3t": np.ascontiguousarray(b3_pad.reshape(ngrp, GRP).T),
    }


def kernel(x, W0, b0, W1, b1, W2, b2, W3, b3):
    x = np.asarray(x, dtype=np.float32)
    W0 = np.asarray(W0, np.float32); b0 = np.asarray(b0, np.float32)
    W1 = np.asarray(W1, np.float32); b1 = np.asarray(b1, np.float32)
    W2 = np.asarray(W2, np.float32); b2 = np.asarray(b2, np.float32)
    W3 = np.asarray(W3, np.float32); b3 = np.asarray(b3, np.float32)

    m_tot = x.shape[0]
    m_loc = m_tot // N_CORES
    nc = _get_nc(m_loc)
    in_maps = [
        _prep_core_inputs(x, W0, b0, W1, b1, W2, b2, W3, b3,
                          slice(c * m_loc, (c + 1) * m_loc))
        for c in range(N_CORES)
    ]
    res = run_bass_kernel_spmd(nc, in_maps, core_ids=list(range(N_CORES)))
    out = np.concatenate([r["y"] for r in res.results], axis=0)
    return out.reshape(m_tot, B, 1).astype(np.float32)


# revision 12
# speedup vs baseline: 1.4644x; 1.3425x over previous
"""TRN2 Bass kernel for nn_EnsemblePointNet: 1296 independent 4-layer MLPs.

Strategy: shard the model dim (1296 -> 162 per core) across 8 NeuronCores.
Per model, activations live transposed ([feature, batch]) so every layer is
one PE matmul with the stored weights as lhsT:
    z = W.T @ h   (lhsT=W [K,M], rhs=h_T [K,B])
fp32r matmuls (full PE rate, ~1.5e-4 rel err). Bias+ReLU fused into the
PSUM->SBUF eviction pass, split between the Scalar (ACT) and Vector (DVE)
engines. DMAs are batched across models (per-DMA fixed cost ~1us dominates
otherwise). The [1,B] layer-3 outputs are evicted to a partition-0 scratch
row shared by XGRP models, DMA'd into a 128-model gather tile, bias-added
there, and stored contiguously.
"""

import contextlib
import sys

sys.path.insert(0, "/opt/trn_rl_repo")

import numpy as np

import concourse.bass as bass
import concourse.mybir as mybir
import concourse.tile as tile
from concourse import bacc
from concourse.bass_utils import run_bass_kernel_spmd

F32 = mybir.dt.float32
F32R = mybir.dt.float32r
AF = mybir.ActivationFunctionType
OP = mybir.AluOpType

M_TOT = 1296
N_CORES = 8
M_LOC = M_TOT // N_CORES  # 162
B = 1024
DIN = 8
H = 128

# ACT is 1.2 GHz, DVE 0.96 GHz; split each relu pass proportionally.
ACT_COLS = 576  # of 1024
HB = 512  # psum bank width in fp32

WGRP = 3   # models per w12 load (w1+w2 packed, 128KB each)
XGRP = 6   # models per xt load / scr evict row
W0GRP = 27  # models per w0 load
GRP = 54   # models per output gather group


def build_nc(m_loc=M_LOC, loop_n=1):
    assert m_loc % GRP == 0 and GRP % XGRP == 0 and GRP % WGRP == 0 and GRP % W0GRP == 0
    nc = bacc.Bacc("TRN2", target_bir_lowering=False, debug=False)
    xt = nc.dram_tensor("xt", [m_loc, DIN, B], F32, kind="ExternalInput").ap()
    w0 = nc.dram_tensor("w0", [m_loc, DIN, H], F32, kind="ExternalInput").ap()
    w12 = nc.dram_tensor("w12", [m_loc, 2, H, H], F32, kind="ExternalInput").ap()
    w3t = nc.dram_tensor("w3t", [H, m_loc], F32, kind="ExternalInput").ap()
    b0t = nc.dram_tensor("b0t", [H, m_loc], F32, kind="ExternalInput").ap()
    b1t = nc.dram_tensor("b1t", [H, m_loc], F32, kind="ExternalInput").ap()
    b2t = nc.dram_tensor("b2t", [H, m_loc], F32, kind="ExternalInput").ap()
    ngrp = m_loc // GRP
    b3t = nc.dram_tensor("b3t", [GRP, ngrp], F32, kind="ExternalInput").ap()
    y = nc.dram_tensor("y", [m_loc, B], F32, kind="ExternalOutput").ap()

    with tile.TileContext(nc) as tc:
        with (
            tc.tile_pool(name="consts", bufs=1) as consts,
            tc.tile_pool(name="wpool", bufs=3) as wpool,
            tc.tile_pool(name="w0pool", bufs=2) as w0pool,
            tc.tile_pool(name="xpool", bufs=2) as xpool,
            tc.tile_pool(name="hpool", bufs=3) as hpool,
            tc.tile_pool(name="ypool", bufs=2) as ypool,
            tc.tile_pool(name="spool", bufs=2) as spool,
            tc.tile_pool(name="zpool", bufs=3, space="PSUM") as zpool,
            tc.tile_pool(name="z3pool", bufs=2, space="PSUM") as z3pool,
        ):
            # one-time constants
            w3t_s = consts.tile([H, m_loc], F32R)
            nc.sync.dma_start(out=w3t_s, in_=w3t.bitcast(F32R))
            b0t_s = consts.tile([H, m_loc], F32)
            nc.sync.dma_start(out=b0t_s, in_=b0t)
            b1t_s = consts.tile([H, m_loc], F32)
            nc.sync.dma_start(out=b1t_s, in_=b1t)
            b2t_s = consts.tile([H, m_loc], F32)
            nc.sync.dma_start(out=b2t_s, in_=b2t)
            b3t_s = consts.tile([GRP, ngrp], F32)
            nc.sync.dma_start(out=b3t_s, in_=b3t)

            def relu_pass(dst, zsrc, bias_ap):
                # dst[128, B] sbuf F32R <- relu(zsrc[128, B] psum + bias)
                nc.scalar.activation(
                    dst[:, 0:ACT_COLS], zsrc[:, 0:ACT_COLS], AF.Relu,
                    bias=bias_ap, scale=1.0,
                )
                nc.vector.tensor_scalar(
                    out=dst[:, ACT_COLS:B], in0=zsrc[:, ACT_COLS:B],
                    scalar1=bias_ap, scalar2=0.0, op0=OP.add, op1=OP.max,
                )

            def body():
                w12s = xts = w0s = scrw = ygat = None

                def load_batches(m):
                    nonlocal w12s, xts, w0s, scrw
                    if m % W0GRP == 0:
                        w0s = w0pool.tile([DIN, W0GRP * H], F32R, tag="w0")
                        nc.sync.dma_start(
                            out=w0s,
                            in_=w0[m : m + W0GRP].rearrange("m i h -> i m h").bitcast(F32R),
                        )
                    if m % WGRP == 0:
                        w12s = wpool.tile([H, WGRP * 2 * H], F32R, tag="w12")
                        nc.sync.dma_start(
                            out=w12s,
                            in_=w12[m : m + WGRP].rearrange("m l h k -> h m l k").bitcast(F32R),
                        )
                    if m % XGRP == 0:
                        xts = xpool.tile([DIN, XGRP * B], F32R, tag="xt")
                        nc.sync.dma_start(
                            out=xts,
                            in_=xt[m : m + XGRP].rearrange("m i b -> i m b").bitcast(F32R),
                        )
                        scrw = spool.tile([1, XGRP * B], F32, tag="scr")

                for g in range(ngrp):
                    g0 = g * GRP
                    ygat = ypool.tile([GRP, B], F32, tag="ygat")
                    for t in range(GRP // 2):
                        ctx = []
                        for m in (g0 + 2 * t, g0 + 2 * t + 1):
                            load_batches(m)
                            ctx.append((m, w12s, xts, w0s, scrw,
                                        (m % XGRP) * B, (m % WGRP) * 2 * H, (m % W0GRP) * H))

                        # stage-interleaved pair: PE never waits on its own model
                        zs = {}
                        for m, ws, xs, w0t, sw, xo, wo, w0o in ctx:
                            z0 = zpool.tile([H, B], F32, tag="z")
                            nc.tensor.matmul(z0[:, 0:HB], w0t[:, w0o : w0o + H], xs[:, xo : xo + HB], start=True, stop=True)
                            nc.tensor.matmul(z0[:, HB:B], w0t[:, w0o : w0o + H], xs[:, xo + HB : xo + B], start=True, stop=True)
                            zs[m] = z0
                        hs = {}
                        for m, ws, xs, w0t, sw, xo, wo, w0o in ctx:
                            h1 = hpool.tile([H, B], F32R, tag="h1")
                            relu_pass(h1, zs[m], b0t_s[:, m : m + 1])
                            hs[m] = h1
                        for m, ws, xs, w0t, sw, xo, wo, w0o in ctx:
                            z1 = zpool.tile([H, B], F32, tag="z")
                            nc.tensor.matmul(z1[:, 0:HB], ws[:, wo : wo + H], hs[m][:, 0:HB], start=True, stop=True)
                            nc.tensor.matmul(z1[:, HB:B], ws[:, wo : wo + H], hs[m][:, HB:B], start=True, stop=True)
                            zs[m] = z1
                        for m, ws, xs, w0t, sw, xo, wo, w0o in ctx:
                            h2 = hpool.tile([H, B], F32R, tag="h2")
                            relu_pass(h2, zs[m], b1t_s[:, m : m + 1])
                            hs[m] = h2
                        for m, ws, xs, w0t, sw, xo, wo, w0o in ctx:
                            z2 = zpool.tile([H, B], F32, tag="z")
                            nc.tensor.matmul(z2[:, 0:HB], ws[:, wo + H : wo + 2 * H], hs[m][:, 0:HB], start=True, stop=True)
                            nc.tensor.matmul(z2[:, HB:B], ws[:, wo + H : wo + 2 * H], hs[m][:, HB:B], start=True, stop=True)
                            zs[m] = z2
                        for m, ws, xs, w0t, sw, xo, wo, w0o in ctx:
                            h3 = hpool.tile([H, B], F32R, tag="h3")
                            relu_pass(h3, zs[m], b2t_s[:, m : m + 1])
                            hs[m] = h3
                        for m, ws, xs, w0t, sw, xo, wo, w0o in ctx:
                            z3a = z3pool.tile([1, HB], F32, tag="z3")
                            z3b = z3pool.tile([1, HB], F32, tag="z3")
                            nc.tensor.matmul(z3a, w3t_s[:, m : m + 1], hs[m][:, 0:HB], start=True, stop=True)
                            nc.tensor.matmul(z3b, w3t_s[:, m : m + 1], hs[m][:, HB:B], start=True, stop=True)
                            xo = (m % XGRP) * B
                            nc.scalar.copy(out=sw[:, xo : xo + HB], in_=z3a)
                            nc.vector.tensor_copy(out=sw[:, xo + HB : xo + B], in_=z3b)
                            if m % XGRP == XGRP - 1:
                                mi = m - g0
                                nc.sync.dma_start(
                                    out=ygat[mi - XGRP + 1 : mi + 1, :], in_=sw
                                )

                    # bias add + store for the group
                    yout = ypool.tile([GRP, B], F32, tag="yout")
                    nc.vector.tensor_scalar(
                        out=yout[0:GRP], in0=ygat[0:GRP],
                        scalar1=b3t_s[0:GRP, g : g + 1], scalar2=None, op0=OP.add,
                    )
                    nc.sync.dma_start(out=y[g0 : g0 + GRP, :], in_=yout[0:GRP])

            if loop_n > 1:
                with tc.For_i(0, loop_n, 1):
                    body()
            else:
                body()

    nc.compile()
    return nc


_NC_CACHE = {}


def _get_nc(m_loc):
    if m_loc not in _NC_CACHE:
        _NC_CACHE[m_loc] = build_nc(m_loc)
    return _NC_CACHE[m_loc]


def _prep_core_inputs(x, W0, b0, W1, b1, W2, b2, W3, b3, sl):
    m_loc = sl.stop - sl.start
    ngrp = m_loc // GRP
    xt = np.ascontiguousarray(np.transpose(x[sl], (0, 2, 1)))  # [m, DIN, B]
    w12 = np.ascontiguousarray(
        np.stack([W1[sl], W2[sl]], axis=1)  # [m, 2, H, H]
    )
    b3_pad = b3[sl, 0].astype(np.float32)
    return {
        "xt": xt,
        "w0": np.ascontiguousarray(W0[sl]),
        "w12": w12,
        "w3t": np.ascontiguousarray(W3[sl, :, 0].T),  # [H, m]
        "b0t": np.ascontiguousarray(b0[sl].T),
        "b1t": np.ascontiguousarray(b1[sl].T),
        "b2t": np.ascontiguousarray(b2[sl].T),
        "b3t": np.ascontiguousarray(b3_pad.reshape(ngrp, GRP).T),
    }


def kernel(x, W0, b0, W1, b1, W2, b2, W3, b3):
    x = np.asarray(x, dtype=np.float32)
    W0 = np.asarray(W0, np.float32); b0 = np.asarray(b0, np.float32)
    W1 = np.asarray(W1, np.float32); b1 = np.asarray(b1, np.float32)
    W2 = np.asarray(W2, np.float32); b2 = np.asarray(b2, np.float32)
    W3 = np.asarray(W3, np.float32); b3 = np.asarray(b3, np.float32)

    m_tot = x.shape[0]
    m_loc = m_tot // N_CORES
    nc = _get_nc(m_loc)
    in_maps = [
        _prep_core_inputs(x, W0, b0, W1, b1, W2, b2, W3, b3,
                          slice(c * m_loc, (c + 1) * m_loc))
        for c in range(N_CORES)
    ]
    res = run_bass_kernel_spmd(nc, in_maps, core_ids=list(range(N_CORES)))
    out = np.concatenate([r["y"] for r in res.results], axis=0)
    return out.reshape(m_tot, B, 1).astype(np.float32)


# revision 13
# speedup vs baseline: 1.8612x; 1.2710x over previous
"""TRN2 Bass kernel for nn_EnsemblePointNet: 1296 independent 4-layer MLPs.

Strategy: shard the model dim (1296 -> 162 per core) across 8 NeuronCores.
Per model, activations live transposed ([feature, batch]) so every layer is
one PE matmul with the stored weights as lhsT:
    z = W.T @ h   (lhsT=W [K,M], rhs=h_T [K,B])
fp32r matmuls (full PE rate, ~1.5e-4 rel err). Bias+ReLU fused into the
PSUM->SBUF eviction pass, split between the Scalar (ACT) and Vector (DVE)
engines. DMAs are batched across models (per-DMA fixed cost ~1us dominates
otherwise). The [1,B] layer-3 outputs are evicted to a partition-0 scratch
row shared by XGRP models, DMA'd into a 128-model gather tile, bias-added
there, and stored contiguously.
"""

import contextlib
import sys

sys.path.insert(0, "/opt/trn_rl_repo")

import numpy as np

import concourse.bass as bass
import concourse.mybir as mybir
import concourse.tile as tile
from concourse import bacc
from concourse.bass_utils import run_bass_kernel_spmd

F32 = mybir.dt.float32
F32R = mybir.dt.float32r
AF = mybir.ActivationFunctionType
OP = mybir.AluOpType

M_TOT = 1296
N_CORES = 8
M_LOC = M_TOT // N_CORES  # 162
B = 1024
DIN = 8
H = 128

# ACT is 1.2 GHz, DVE 0.96 GHz; split each relu pass proportionally.
ACT_COLS = 640  # of 1024
HB = 512  # psum bank width in fp32

WGRP = 3   # models per w12 load (w1+w2 packed, 128KB each)
XGRP = 6   # models per xt load / scr evict row
W0GRP = 18  # models per w0 load
GRP = 54   # models per output gather group
ILV = 3    # models interleaved per pipeline round


def build_nc(m_loc=M_LOC, loop_n=1):
    assert m_loc % GRP == 0 and GRP % XGRP == 0 and GRP % WGRP == 0 and GRP % W0GRP == 0
    nc = bacc.Bacc("TRN2", target_bir_lowering=False, debug=False)
    xt = nc.dram_tensor("xt", [m_loc, DIN, B], F32, kind="ExternalInput").ap()
    w0 = nc.dram_tensor("w0", [m_loc, DIN, H], F32, kind="ExternalInput").ap()
    w12 = nc.dram_tensor("w12", [m_loc, 2, H, H], F32, kind="ExternalInput").ap()
    w3t = nc.dram_tensor("w3t", [H, m_loc], F32, kind="ExternalInput").ap()
    b0t = nc.dram_tensor("b0t", [H, m_loc], F32, kind="ExternalInput").ap()
    b1t = nc.dram_tensor("b1t", [H, m_loc], F32, kind="ExternalInput").ap()
    b2t = nc.dram_tensor("b2t", [H, m_loc], F32, kind="ExternalInput").ap()
    ngrp = m_loc // GRP
    b3t = nc.dram_tensor("b3t", [GRP, ngrp], F32, kind="ExternalInput").ap()
    y = nc.dram_tensor("y", [m_loc, B], F32, kind="ExternalOutput").ap()

    with tile.TileContext(nc) as tc:
        with (
            tc.tile_pool(name="consts", bufs=1) as consts,
            tc.tile_pool(name="wpool", bufs=3) as wpool,
            tc.tile_pool(name="w0pool", bufs=2) as w0pool,
            tc.tile_pool(name="xpool", bufs=2) as xpool,
            tc.tile_pool(name="hpool", bufs=3) as hpool,
            tc.tile_pool(name="ypool", bufs=2) as ypool,
            tc.tile_pool(name="spool", bufs=2) as spool,
            tc.tile_pool(name="zpool", bufs=3, space="PSUM") as zpool,
            tc.tile_pool(name="z3pool", bufs=2, space="PSUM") as z3pool,
        ):
            # one-time constants
            w3t_s = consts.tile([H, m_loc], F32R)
            nc.sync.dma_start(out=w3t_s, in_=w3t.bitcast(F32R))
            b0t_s = consts.tile([H, m_loc], F32)
            nc.sync.dma_start(out=b0t_s, in_=b0t)
            b1t_s = consts.tile([H, m_loc], F32)
            nc.sync.dma_start(out=b1t_s, in_=b1t)
            b2t_s = consts.tile([H, m_loc], F32)
            nc.sync.dma_start(out=b2t_s, in_=b2t)
            b3t_s = consts.tile([GRP, ngrp], F32)
            nc.sync.dma_start(out=b3t_s, in_=b3t)

            def relu_pass(dst, zsrc, bias_ap):
                # dst[128, B] sbuf F32R <- relu(zsrc[128, B] psum + bias)
                nc.scalar.activation(
                    dst[:, 0:ACT_COLS], zsrc[:, 0:ACT_COLS], AF.Relu,
                    bias=bias_ap, scale=1.0,
                )
                nc.vector.tensor_scalar(
                    out=dst[:, ACT_COLS:B], in0=zsrc[:, ACT_COLS:B],
                    scalar1=bias_ap, scalar2=0.0, op0=OP.add, op1=OP.max,
                )

            def body():
                w12s = xts = w0s = scrw = ygat = None

                def load_batches(m):
                    nonlocal w12s, xts, w0s, scrw
                    if m % W0GRP == 0:
                        w0s = w0pool.tile([DIN, W0GRP * H], F32R, tag="w0")
                        nc.sync.dma_start(
                            out=w0s,
                            in_=w0[m : m + W0GRP].rearrange("m i h -> i m h").bitcast(F32R),
                        )
                    if m % WGRP == 0:
                        w12s = wpool.tile([H, WGRP * 2 * H], F32R, tag="w12")
                        nc.sync.dma_start(
                            out=w12s,
                            in_=w12[m : m + WGRP].rearrange("m l h k -> h m l k").bitcast(F32R),
                        )
                    if m % XGRP == 0:
                        xts = xpool.tile([DIN, XGRP * B], F32R, tag="xt")
                        nc.sync.dma_start(
                            out=xts,
                            in_=xt[m : m + XGRP].rearrange("m i b -> i m b").bitcast(F32R),
                        )
                        scrw = spool.tile([1, XGRP * B], F32, tag="scr")

                for g in range(ngrp):
                    g0 = g * GRP
                    ygat = ypool.tile([GRP, B], F32, tag="ygat")
                    for t in range(GRP // ILV):
                        ctx = []
                        for m in range(g0 + ILV * t, g0 + ILV * t + ILV):
                            load_batches(m)
                            ctx.append((m, w12s, xts, w0s, scrw,
                                        (m % XGRP) * B, (m % WGRP) * 2 * H, (m % W0GRP) * H))

                        # stage-interleaved pair: PE never waits on its own model
                        zs = {}
                        for m, ws, xs, w0t, sw, xo, wo, w0o in ctx:
                            z0 = zpool.tile([H, B], F32, tag="z")
                            nc.tensor.matmul(z0[:, 0:HB], w0t[:, w0o : w0o + H], xs[:, xo : xo + HB], start=True, stop=True)
                            nc.tensor.matmul(z0[:, HB:B], w0t[:, w0o : w0o + H], xs[:, xo + HB : xo + B], start=True, stop=True)
                            zs[m] = z0
                        hs = {}
                        for m, ws, xs, w0t, sw, xo, wo, w0o in ctx:
                            h1 = hpool.tile([H, B], F32R, tag="h1")
                            relu_pass(h1, zs[m], b0t_s[:, m : m + 1])
                            hs[m] = h1
                        for m, ws, xs, w0t, sw, xo, wo, w0o in ctx:
                            z1 = zpool.tile([H, B], F32, tag="z")
                            nc.tensor.matmul(z1[:, 0:HB], ws[:, wo : wo + H], hs[m][:, 0:HB], start=True, stop=True)
                            nc.tensor.matmul(z1[:, HB:B], ws[:, wo : wo + H], hs[m][:, HB:B], start=True, stop=True)
                            zs[m] = z1
                        for m, ws, xs, w0t, sw, xo, wo, w0o in ctx:
                            h2 = hpool.tile([H, B], F32R, tag="h2")
                            relu_pass(h2, zs[m], b1t_s[:, m : m + 1])
                            hs[m] = h2
                        for m, ws, xs, w0t, sw, xo, wo, w0o in ctx:
                            z2 = zpool.tile([H, B], F32, tag="z")
                            nc.tensor.matmul(z2[:, 0:HB], ws[:, wo + H : wo + 2 * H], hs[m][:, 0:HB], start=True, stop=True)
                            nc.tensor.matmul(z2[:, HB:B], ws[:, wo + H : wo + 2 * H], hs[m][:, HB:B], start=True, stop=True)
                            zs[m] = z2
                        for m, ws, xs, w0t, sw, xo, wo, w0o in ctx:
                            h3 = hpool.tile([H, B], F32R, tag="h3")
                            relu_pass(h3, zs[m], b2t_s[:, m : m + 1])
                            hs[m] = h3
                        for m, ws, xs, w0t, sw, xo, wo, w0o in ctx:
                            z3a = z3pool.tile([1, HB], F32, tag="z3")
                            z3b = z3pool.tile([1, HB], F32, tag="z3")
                            nc.tensor.matmul(z3a, w3t_s[:, m : m + 1], hs[m][:, 0:HB], start=True, stop=True)
                            nc.tensor.matmul(z3b, w3t_s[:, m : m + 1], hs[m][:, HB:B], start=True, stop=True)
                            xo = (m % XGRP) * B
                            nc.scalar.copy(out=sw[:, xo : xo + HB], in_=z3a)
                            nc.vector.tensor_copy(out=sw[:, xo + HB : xo + B], in_=z3b)
                            if m % XGRP == XGRP - 1:
                                mi = m - g0
                                nc.sync.dma_start(
                                    out=ygat[mi - XGRP + 1 : mi + 1, :], in_=sw
                                )

                    # bias add + store for the group
                    yout = ypool.tile([GRP, B], F32, tag="yout")
                    nc.vector.tensor_scalar(
                        out=yout[0:GRP], in0=ygat[0:GRP],
                        scalar1=b3t_s[0:GRP, g : g + 1], scalar2=None, op0=OP.add,
                    )
                    nc.sync.dma_start(out=y[g0 : g0 + GRP, :], in_=yout[0:GRP])

            if loop_n > 1:
                with tc.For_i(0, loop_n, 1):
                    body()
            else:
                body()

    nc.compile()
    return nc


_NC_CACHE = {}


def _get_nc(m_loc):
    if m_loc not in _NC_CACHE:
        _NC_CACHE[m_loc] = build_nc(m_loc)
    return _NC_CACHE[m_loc]


def _prep_core_inputs(x, W0, b0, W1, b1, W2, b2, W3, b3, sl):
    m_loc = sl.stop - sl.start
    ngrp = m_loc // GRP
    xt = np.ascontiguousarray(np.transpose(x[sl], (0, 2, 1)))  # [m, DIN, B]
    w12 = np.ascontiguousarray(
        np.stack([W1[sl], W2[sl]], axis=1)  # [m, 2, H, H]
    )
    b3_pad = b3[sl, 0].astype(np.float32)
    return {
        "xt": xt,
        "w0": np.ascontiguousarray(W0[sl]),
        "w12": w12,
        "w3t": np.ascontiguousarray(W3[sl, :, 0].T),  # [H, m]
        "b0t": np.ascontiguousarray(b0[sl].T),
        "b1t": np.ascontiguousarray(b1[sl].T),
        "b2t": np.ascontiguousarray(b2[sl].T),
        "b3t": np.ascontiguousarray(b3_pad.reshape(ngrp, GRP).T),
    }


def kernel(x, W0, b0, W1, b1, W2, b2, W3, b3):
    x = np.asarray(x, dtype=np.float32)
    W0 = np.asarray(W0, np.float32); b0 = np.asarray(b0, np.float32)
    W1 = np.asarray(W1, np.float32); b1 = np.asarray(b1, np.float32)
    W2 = np.asarray(W2, np.float32); b2 = np.asarray(b2, np.float32)
    W3 = np.asarray(W3, np.float32); b3 = np.asarray(b3, np.float32)

    m_tot = x.shape[0]
    m_loc = m_tot // N_CORES
    nc = _get_nc(m_loc)
    in_maps = [
        _prep_core_inputs(x, W0, b0, W1, b1, W2, b2, W3, b3,
                          slice(c * m_loc, (c + 1) * m_loc))
        for c in range(N_CORES)
    ]
    res = run_bass_kernel_spmd(nc, in_maps, core_ids=list(range(N_CORES)))
    out = np.concatenate([r["y"] for r in res.results], axis=0)
    return out.reshape(m_tot, B, 1).astype(np.float32)


# revision 15
# speedup vs baseline: 2.2819x; 1.2261x over previous
"""TRN2 Bass kernel for nn_EnsemblePointNet: 1296 independent 4-layer MLPs.

Strategy: shard the model dim (1296 -> 162 per core) across 8 NeuronCores.
Per model, activations live transposed ([feature, batch]) so every layer is
one PE matmul with the stored weights as lhsT:
    z = W.T @ h   (lhsT=W [K,M], rhs=h_T [K,B])
fp32r matmuls (full PE rate, ~1.5e-4 rel err). Bias+ReLU fused into the
PSUM->SBUF eviction pass, split between the Scalar (ACT) and Vector (DVE)
engines. DMAs are batched across models (per-DMA fixed cost ~1us dominates
otherwise). The [1,B] layer-3 outputs are evicted to a partition-0 scratch
row shared by XGRP models, DMA'd into a 128-model gather tile, bias-added
there, and stored contiguously.
"""

import contextlib
import sys

sys.path.insert(0, "/opt/trn_rl_repo")

import numpy as np

import concourse.bass as bass
import concourse.mybir as mybir
import concourse.tile as tile
from concourse import bacc
from concourse.bass_utils import run_bass_kernel_spmd

F32 = mybir.dt.float32
F32R = mybir.dt.float32r
F16 = mybir.dt.float16
AF = mybir.ActivationFunctionType
OP = mybir.AluOpType

M_TOT = 1296
N_CORES = 8
M_LOC = M_TOT // N_CORES  # 162
B = 1024
DIN = 8
H = 128

# ACT is 1.2 GHz, DVE 0.96 GHz; split each relu pass proportionally.
ACT_COLS = 640  # of 1024
HB = 512  # psum bank width in fp32

WGRP = 3   # models per w12 load (w1+w2 packed, 128KB each)
XGRP = 6   # models per xt load / scr evict row
W0GRP = 18  # models per w0 load
GRP = 54   # models per output gather group
ILV = 4    # models interleaved per pipeline round (quad for col-tiled L3)


def build_nc(m_loc=M_LOC, loop_n=1):
    assert m_loc % GRP == 0 and GRP % XGRP == 0 and GRP % WGRP == 0 and GRP % W0GRP == 0
    nc = bacc.Bacc("TRN2", target_bir_lowering=False, debug=False)
    xt = nc.dram_tensor("xt", [m_loc, DIN, B], F32, kind="ExternalInput").ap()
    w0 = nc.dram_tensor("w0", [m_loc, DIN, H], F32, kind="ExternalInput").ap()
    w12 = nc.dram_tensor("w12", [m_loc, 2, H, H], F32, kind="ExternalInput").ap()
    w3t16 = nc.dram_tensor("w3t16", [H, m_loc + 31], F16, kind="ExternalInput").ap()
    b0t = nc.dram_tensor("b0t", [H, m_loc], F32, kind="ExternalInput").ap()
    b1t = nc.dram_tensor("b1t", [H, m_loc], F32, kind="ExternalInput").ap()
    b2t = nc.dram_tensor("b2t", [H, m_loc], F32, kind="ExternalInput").ap()
    ngrp = m_loc // GRP
    b3t = nc.dram_tensor("b3t", [GRP, ngrp], F32, kind="ExternalInput").ap()
    y = nc.dram_tensor("y", [m_loc, B], F32, kind="ExternalOutput").ap()

    with tile.TileContext(nc) as tc:
        with (
            tc.tile_pool(name="consts", bufs=1) as consts,
            tc.tile_pool(name="wpool", bufs=4) as wpool,
            tc.tile_pool(name="w0pool", bufs=2) as w0pool,
            tc.tile_pool(name="xpool", bufs=3) as xpool,
            tc.tile_pool(name="hpool", bufs=5) as hpool,
            tc.tile_pool(name="ypool", bufs=2) as ypool,
            tc.tile_pool(name="spool", bufs=2) as spool,
            tc.tile_pool(name="zpool", bufs=6, space="PSUM") as zpool,
            tc.tile_pool(name="zqpool", bufs=1, space="PSUM") as zqpool,
        ):
            # one-time constants
            w3t_s = consts.tile([H, m_loc + 31], F16)
            nc.sync.dma_start(out=w3t_s, in_=w3t16)
            b0t_s = consts.tile([H, m_loc], F32)
            nc.sync.dma_start(out=b0t_s, in_=b0t)
            b1t_s = consts.tile([H, m_loc], F32)
            nc.sync.dma_start(out=b1t_s, in_=b1t)
            b2t_s = consts.tile([H, m_loc], F32)
            nc.sync.dma_start(out=b2t_s, in_=b2t)
            b3t_s = consts.tile([GRP, ngrp], F32)
            nc.sync.dma_start(out=b3t_s, in_=b3t)

            def relu_half(dst_half, z_half, bias_ap, on_act):
                # dst[128, HB] sbuf <- relu(z[128, HB] psum + bias)
                if on_act:
                    nc.scalar.activation(
                        dst_half, z_half, AF.Relu, bias=bias_ap, scale=1.0,
                    )
                else:
                    nc.vector.tensor_scalar(
                        out=dst_half, in0=z_half,
                        scalar1=bias_ap, scalar2=0.0, op0=OP.add, op1=OP.max,
                    )

            def body():
                w12s = xts = w0s = scrw = ygat = None

                def load_batches(m):
                    nonlocal w12s, xts, w0s, scrw
                    if m % W0GRP == 0:
                        w0s = w0pool.tile([DIN, W0GRP * H], F32R, tag="w0")
                        nc.sync.dma_start(
                            out=w0s,
                            in_=w0[m : m + W0GRP].rearrange("m i h -> i m h").bitcast(F32R),
                        )
                    if m % WGRP == 0:
                        w12s = wpool.tile([H, WGRP * 2 * H], F32R, tag="w12")
                        nc.sync.dma_start(
                            out=w12s,
                            in_=w12[m : m + WGRP].rearrange("m l h k -> h m l k").bitcast(F32R),
                        )
                    if m % XGRP == 0:
                        xts = xpool.tile([DIN, XGRP * B], F32R, tag="xt")
                        nc.sync.dma_start(
                            out=xts,
                            in_=xt[m : m + XGRP].rearrange("m i b -> i m b").bitcast(F32R),
                        )

                for g in range(ngrp):
                    g0 = g * GRP
                    ygat = ypool.tile([GRP, B], F32, tag="ygat")
                    rounds = [list(range(g0 + r, min(g0 + r + ILV, g0 + GRP)))
                              for r in range(0, GRP, ILV)]
                    for models in rounds:
                        ctx = {}
                        for m in models:
                            load_batches(m)
                            ctx[m] = (w12s, xts, w0s,
                                      (m % XGRP) * B, (m % WGRP) * 2 * H, (m % W0GRP) * H)

                        def mm_layer(lsel, rhs_of, zdict):
                            for m in models:
                                ws_, xs_, w0_, xo, wo, w0o = ctx[m]
                                za = zpool.tile([H, HB], F32, tag="z")
                                zb = zpool.tile([H, HB], F32, tag="z")
                                if lsel == 0:
                                    lhs = w0_[:, w0o : w0o + H]
                                    ra = xs_[:, xo : xo + HB]
                                    rb = xs_[:, xo + HB : xo + B]
                                else:
                                    lhs = ws_[:, wo + (lsel - 1) * H : wo + lsel * H]
                                    h = rhs_of[m]
                                    ra = h[:, 0:HB]
                                    rb = h[:, HB:B]
                                nc.tensor.matmul(za, lhs, ra, start=True, stop=True)
                                nc.tensor.matmul(zb, lhs, rb, start=True, stop=True)
                                zdict[m] = (za, zb)

                        def relu_layer(zdict, bias_t, tag, dt, hdict):
                            for m in models:
                                za, zb = zdict[m]
                                h = hpool.tile([H, B], dt, tag=tag)
                                relu_half(h[:, 0:HB], za, bias_t[:, m : m + 1], True)
                                relu_half(h[:, HB:B], zb, bias_t[:, m : m + 1], False)
                                hdict[m] = h

                        zs, hs = {}, {}
                        mm_layer(0, None, zs)
                        relu_layer(zs, b0t_s, "h1", F32R, hs)
                        zs = {}
                        mm_layer(1, hs, zs)
                        h2s = {}
                        relu_layer(zs, b1t_s, "h2", F32R, h2s)
                        zs = {}
                        mm_layer(2, h2s, zs)
                        h3s = {}
                        relu_layer(zs, b2t_s, "h3", F16, h3s)

                        # col-tiled quad L3: model j -> psum rows 32j
                        zq = zqpool.tile([128, B], F32, tag="zq")
                        for j, m in enumerate(models):
                            nc.tensor.matmul(
                                zq[32 * j : 32 * j + 32, 0:HB],
                                w3t_s[:, m : m + 32], h3s[m][:, 0:HB],
                                start=True, stop=True, tile_position=(0, 32 * j),
                            )
                            nc.tensor.matmul(
                                zq[32 * j : 32 * j + 32, HB:B],
                                w3t_s[:, m : m + 32], h3s[m][:, HB:B],
                                start=True, stop=True, tile_position=(0, 32 * j),
                            )
                        scr = spool.tile([128, B], F32, tag="scr")
                        nc.scalar.copy(out=scr[:, 0:HB], in_=zq[:, 0:HB])
                        nc.vector.tensor_copy(out=scr[:, HB:B], in_=zq[:, HB:B])
                        mi0 = models[0] - g0
                        nq = len(models)
                        sv = scr.rearrange("(a p) b -> a p b", a=4)[0:nq, 0, :]
                        nc.sync.dma_start(out=ygat[mi0 : mi0 + nq, :], in_=sv)

                    # bias add + store for the group
                    yout = ypool.tile([GRP, B], F32, tag="yout")
                    nc.vector.tensor_scalar(
                        out=yout[0:GRP], in0=ygat[0:GRP],
                        scalar1=b3t_s[0:GRP, g : g + 1], scalar2=None, op0=OP.add,
                    )
                    nc.sync.dma_start(out=y[g0 : g0 + GRP, :], in_=yout[0:GRP])

            if loop_n > 1:
                with tc.For_i(0, loop_n, 1):
                    body()
            else:
                body()

    nc.compile()
    return nc


_NC_CACHE = {}


def _get_nc(m_loc):
    if m_loc not in _NC_CACHE:
        _NC_CACHE[m_loc] = build_nc(m_loc)
    return _NC_CACHE[m_loc]


def _prep_core_inputs(x, W0, b0, W1, b1, W2, b2, W3, b3, sl):
    m_loc = sl.stop - sl.start
    ngrp = m_loc // GRP
    xt = np.ascontiguousarray(np.transpose(x[sl], (0, 2, 1)))  # [m, DIN, B]
    w12 = np.ascontiguousarray(
        np.stack([W1[sl], W2[sl]], axis=1)  # [m, 2, H, H]
    )
    b3_pad = b3[sl, 0].astype(np.float32)
    return {
        "xt": xt,
        "w0": np.ascontiguousarray(W0[sl]),
        "w12": w12,
        "w3t16": np.ascontiguousarray(
            np.pad(W3[sl, :, 0], ((0, 31), (0, 0))).T.astype(np.float16)
        ),  # [H, m+31]
        "b0t": np.ascontiguousarray(b0[sl].T),
        "b1t": np.ascontiguousarray(b1[sl].T),
        "b2t": np.ascontiguousarray(b2[sl].T),
        "b3t": np.ascontiguousarray(b3_pad.reshape(ngrp, GRP).T),
    }


def kernel(x, W0, b0, W1, b1, W2, b2, W3, b3):
    x = np.asarray(x, dtype=np.float32)
    W0 = np.asarray(W0, np.float32); b0 = np.asarray(b0, np.float32)
    W1 = np.asarray(W1, np.float32); b1 = np.asarray(b1, np.float32)
    W2 = np.asarray(W2, np.float32); b2 = np.asarray(b2, np.float32)
    W3 = np.asarray(W3, np.float32); b3 = np.asarray(b3, np.float32)

    m_tot = x.shape[0]
    m_loc = m_tot // N_CORES
    nc = _get_nc(m_loc)
    in_maps = [
        _prep_core_inputs(x, W0, b0, W1, b1, W2, b2, W3, b3,
                          slice(c * m_loc, (c + 1) * m_loc))
        for c in range(N_CORES)
    ]
    res = run_bass_kernel_spmd(nc, in_maps, core_ids=list(range(N_CORES)))
    out = np.concatenate([r["y"] for r in res.results], axis=0)
    return out.reshape(m_tot, B, 1).astype(np.float32)


# revision 16
# speedup vs baseline: 2.3411x; 1.0259x over previous
"""TRN2 Bass kernel for nn_EnsemblePointNet: 1296 independent 4-layer MLPs.

Strategy: shard the model dim (1296 -> 162 per core) across 8 NeuronCores.
Per model, activations live transposed ([feature, batch]) so every layer is
one PE matmul with the stored weights as lhsT:
    z = W.T @ h   (lhsT=W [K,M], rhs=h_T [K,B])
fp32r matmuls (full PE rate, ~1.5e-4 rel err). Bias+ReLU fused into the
PSUM->SBUF eviction pass, split between the Scalar (ACT) and Vector (DVE)
engines. DMAs are batched across models (per-DMA fixed cost ~1us dominates
otherwise). The [1,B] layer-3 outputs are evicted to a partition-0 scratch
row shared by XGRP models, DMA'd into a 128-model gather tile, bias-added
there, and stored contiguously.
"""

import contextlib
import sys

sys.path.insert(0, "/opt/trn_rl_repo")

import numpy as np

import concourse.bass as bass
import concourse.mybir as mybir
import concourse.tile as tile
from concourse import bacc
from concourse.bass_utils import run_bass_kernel_spmd

F32 = mybir.dt.float32
F32R = mybir.dt.float32r
F16 = mybir.dt.float16
AF = mybir.ActivationFunctionType
OP = mybir.AluOpType

M_TOT = 1296
N_CORES = 8
M_LOC = M_TOT // N_CORES  # 162
B = 1024
DIN = 8
H = 128

# ACT is 1.2 GHz, DVE 0.96 GHz; split each relu pass proportionally.
ACT_COLS = 640  # of 1024
HB = 512  # psum bank width in fp32

WGRP = 3   # models per w12 load (w1+w2 packed, 128KB each)
XGRP = 6   # models per xt load / scr evict row
W0GRP = 18  # models per w0 load
GRP = 54   # models per output gather group
ILV = 4    # models interleaved per pipeline round (quad for col-tiled L3)


def build_nc(m_loc=M_LOC, loop_n=1):
    assert m_loc % GRP == 0 and GRP % XGRP == 0 and GRP % WGRP == 0 and GRP % W0GRP == 0
    nc = bacc.Bacc("TRN2", target_bir_lowering=False, debug=False)
    xt = nc.dram_tensor("xt", [m_loc, DIN, B], F32, kind="ExternalInput").ap()
    w0 = nc.dram_tensor("w0", [m_loc, DIN, H], F32, kind="ExternalInput").ap()
    w12 = nc.dram_tensor("w12", [m_loc, 2, H, H], F32, kind="ExternalInput").ap()
    w3t16 = nc.dram_tensor("w3t16", [H, m_loc + 31], F16, kind="ExternalInput").ap()
    b0t = nc.dram_tensor("b0t", [H, m_loc], F32, kind="ExternalInput").ap()
    b1t = nc.dram_tensor("b1t", [H, m_loc], F32, kind="ExternalInput").ap()
    b2t = nc.dram_tensor("b2t", [H, m_loc], F32, kind="ExternalInput").ap()
    ngrp = m_loc // GRP
    b3t = nc.dram_tensor("b3t", [GRP, ngrp], F32, kind="ExternalInput").ap()
    y = nc.dram_tensor("y", [m_loc, B], F32, kind="ExternalOutput").ap()

    with tile.TileContext(nc) as tc:
        with (
            tc.tile_pool(name="consts", bufs=1) as consts,
            tc.tile_pool(name="wpool", bufs=4) as wpool,
            tc.tile_pool(name="w0pool", bufs=2) as w0pool,
            tc.tile_pool(name="xpool", bufs=3) as xpool,
            tc.tile_pool(name="hpool", bufs=5) as hpool,
            tc.tile_pool(name="ypool", bufs=2) as ypool,
            tc.tile_pool(name="spool", bufs=2) as spool,
            tc.tile_pool(name="zpool", bufs=6, space="PSUM") as zpool,
            tc.tile_pool(name="zqpool", bufs=2, space="PSUM") as zqpool,
        ):
            # one-time constants
            w3t_s = consts.tile([H, m_loc + 31], F16)
            nc.sync.dma_start(out=w3t_s, in_=w3t16)
            b0t_s = consts.tile([H, m_loc], F32)
            nc.sync.dma_start(out=b0t_s, in_=b0t)
            b1t_s = consts.tile([H, m_loc], F32)
            nc.sync.dma_start(out=b1t_s, in_=b1t)
            b2t_s = consts.tile([H, m_loc], F32)
            nc.sync.dma_start(out=b2t_s, in_=b2t)
            b3t_s = consts.tile([GRP, ngrp], F32)
            nc.sync.dma_start(out=b3t_s, in_=b3t)

            def relu_half(dst_half, z_half, bias_ap, on_act):
                # dst[128, HB] sbuf <- relu(z[128, HB] psum + bias)
                if on_act:
                    nc.scalar.activation(
                        dst_half, z_half, AF.Relu, bias=bias_ap, scale=1.0,
                    )
                else:
                    nc.vector.tensor_scalar(
                        out=dst_half, in0=z_half,
                        scalar1=bias_ap, scalar2=0.0, op0=OP.add, op1=OP.max,
                    )

            def body():
                w12s = xts = w0s = scrw = ygat = None

                def load_batches(m):
                    nonlocal w12s, xts, w0s, scrw
                    if m % W0GRP == 0:
                        w0s = w0pool.tile([DIN, W0GRP * H], F32R, tag="w0")
                        nc.sync.dma_start(
                            out=w0s,
                            in_=w0[m : m + W0GRP].rearrange("m i h -> i m h").bitcast(F32R),
                        )
                    if m % WGRP == 0:
                        w12s = wpool.tile([H, WGRP * 2 * H], F32R, tag="w12")
                        nc.sync.dma_start(
                            out=w12s,
                            in_=w12[m : m + WGRP].rearrange("m l h k -> h m l k").bitcast(F32R),
                        )
                    if m % XGRP == 0:
                        xts = xpool.tile([DIN, XGRP * B], F32R, tag="xt")
                        nc.sync.dma_start(
                            out=xts,
                            in_=xt[m : m + XGRP].rearrange("m i b -> i m b").bitcast(F32R),
                        )

                for g in range(ngrp):
                    g0 = g * GRP
                    ygat = ypool.tile([GRP, B], F32, tag="ygat")
                    rounds = [list(range(g0 + r, min(g0 + r + ILV, g0 + GRP)))
                              for r in range(0, GRP, ILV)]
                    for models in rounds:
                        ctx = {}
                        for m in models:
                            load_batches(m)
                            ctx[m] = (w12s, xts, w0s,
                                      (m % XGRP) * B, (m % WGRP) * 2 * H, (m % W0GRP) * H)

                        def mm_layer(lsel, rhs_of, zdict):
                            for m in models:
                                ws_, xs_, w0_, xo, wo, w0o = ctx[m]
                                za = zpool.tile([H, HB], F32, tag="z")
                                zb = zpool.tile([H, HB], F32, tag="z")
                                if lsel == 0:
                                    lhs = w0_[:, w0o : w0o + H]
                                    ra = xs_[:, xo : xo + HB]
                                    rb = xs_[:, xo + HB : xo + B]
                                else:
                                    lhs = ws_[:, wo + (lsel - 1) * H : wo + lsel * H]
                                    h = rhs_of[m]
                                    ra = h[:, 0:HB]
                                    rb = h[:, HB:B]
                                nc.tensor.matmul(za, lhs, ra, start=True, stop=True)
                                nc.tensor.matmul(zb, lhs, rb, start=True, stop=True)
                                zdict[m] = (za, zb)

                        def relu_layer(zdict, bias_t, tag, dt, hdict):
                            for m in models:
                                za, zb = zdict[m]
                                h = hpool.tile([H, B], dt, tag=tag)
                                relu_half(h[:, 0:HB], za, bias_t[:, m : m + 1], True)
                                relu_half(h[:, HB:B], zb, bias_t[:, m : m + 1], False)
                                hdict[m] = h

                        zs, hs = {}, {}
                        mm_layer(0, None, zs)
                        relu_layer(zs, b0t_s, "h1", F32R, hs)
                        zs = {}
                        mm_layer(1, hs, zs)
                        h2s = {}
                        relu_layer(zs, b1t_s, "h2", F32R, h2s)
                        zs = {}
                        mm_layer(2, h2s, zs)
                        h3s = {}
                        relu_layer(zs, b2t_s, "h3", F16, h3s)

                        # col-tiled quad L3: model j -> psum rows 32j
                        zqa = zqpool.tile([128, HB], F32, tag="zq")
                        zqb = zqpool.tile([128, HB], F32, tag="zq")
                        for j, m in enumerate(models):
                            nc.tensor.matmul(
                                zqa[32 * j : 32 * j + 32, :],
                                w3t_s[:, m : m + 32], h3s[m][:, 0:HB],
                                start=True, stop=True, tile_position=(0, 32 * j),
                            )
                            nc.tensor.matmul(
                                zqb[32 * j : 32 * j + 32, :],
                                w3t_s[:, m : m + 32], h3s[m][:, HB:B],
                                start=True, stop=True, tile_position=(0, 32 * j),
                            )
                        scr = spool.tile([128, B], F32, tag="scr")
                        nc.scalar.copy(out=scr[:, 0:HB], in_=zqa)
                        nc.scalar.copy(out=scr[:, HB:B], in_=zqb)
                        mi0 = models[0] - g0
                        nq = len(models)
                        sv = scr.rearrange("(a p) b -> a p b", a=4)[0:nq, 0, :]
                        nc.sync.dma_start(out=ygat[mi0 : mi0 + nq, :], in_=sv)

                    # bias add + store for the group
                    yout = ypool.tile([GRP, B], F32, tag="yout")
                    nc.scalar.add(yout[0:GRP], ygat[0:GRP], b3t_s[0:GRP, g : g + 1])
                    nc.sync.dma_start(out=y[g0 : g0 + GRP, :], in_=yout[0:GRP])

            if loop_n > 1:
                with tc.For_i(0, loop_n, 1):
                    body()
            else:
                body()

    nc.compile()
    return nc


_NC_CACHE = {}


def _get_nc(m_loc):
    if m_loc not in _NC_CACHE:
        _NC_CACHE[m_loc] = build_nc(m_loc)
    return _NC_CACHE[m_loc]


def _prep_core_inputs(x, W0, b0, W1, b1, W2, b2, W3, b3, sl):
    m_loc = sl.stop - sl.start
    ngrp = m_loc // GRP
    xt = np.ascontiguousarray(np.transpose(x[sl], (0, 2, 1)))  # [m, DIN, B]
    w12 = np.ascontiguousarray(
        np.stack([W1[sl], W2[sl]], axis=1)  # [m, 2, H, H]
    )
    b3_pad = b3[sl, 0].astype(np.float32)
    return {
        "xt": xt,
        "w0": np.ascontiguousarray(W0[sl]),
        "w12": w12,
        "w3t16": np.ascontiguousarray(
            np.pad(W3[sl, :, 0], ((0, 31), (0, 0))).T.astype(np.float16)
        ),  # [H, m+31]
        "b0t": np.ascontiguousarray(b0[sl].T),
        "b1t": np.ascontiguousarray(b1[sl].T),
        "b2t": np.ascontiguousarray(b2[sl].T),
        "b3t": np.ascontiguousarray(b3_pad.reshape(ngrp, GRP).T),
    }


def kernel(x, W0, b0, W1, b1, W2, b2, W3, b3):
    x = np.asarray(x, dtype=np.float32)
    W0 = np.asarray(W0, np.float32); b0 = np.asarray(b0, np.float32)
    W1 = np.asarray(W1, np.float32); b1 = np.asarray(b1, np.float32)
    W2 = np.asarray(W2, np.float32); b2 = np.asarray(b2, np.float32)
    W3 = np.asarray(W3, np.float32); b3 = np.asarray(b3, np.float32)

    m_tot = x.shape[0]
    m_loc = m_tot // N_CORES
    nc = _get_nc(m_loc)
    in_maps = [
        _prep_core_inputs(x, W0, b0, W1, b1, W2, b2, W3, b3,
                          slice(c * m_loc, (c + 1) * m_loc))
        for c in range(N_CORES)
    ]
    res = run_bass_kernel_spmd(nc, in_maps, core_ids=list(range(N_CORES)))
    out = np.concatenate([r["y"] for r in res.results], axis=0)
    return out.reshape(m_tot, B, 1).astype(np.float32)
